# revision 51
# baseline (speedup 1.0000x reference)
"""FlowNet-C correlation (max_displacement=4) on 8 Trainium2 NeuronCores.

Strategy: data-parallel over batch N=8 (one sample per core).
Per core: out[d=(dy,dx), y, x] = 1/C * sum_c in1[c,y,x] * in2pad[c,y+dy,x+dx]

Mapping: the correlation is a banded Gram. For each 8x16 spatial block of
in1 (M=128 positions, host-pre-tiled to be SBUF-contiguous) we matmul
(contract c=256 in 2 K-halves) against the 16x24 padded window of in2
(N=384 columns) in bf16 (full-rate; inputs are bf16-converted on host,
in2 horizontally pre-padded to 136 cols so every DMA chunk is >=512B
contiguous). Two adjacent blocks share a 2-bank PSUM tile; ScalarE /
VectorE (alternating) evacuate with a fused 1/C scale + bf16 downcast.
Band extraction (81 of 384 Gram cols per position, per-partition offsets)
is split: the first K_BAND pair-tiles go through GPSIMD local_scatter on
device; the last K_RAW pairs ship their raw Gram tile and the host
extracts, balancing the Pool engine against the serial DMA resource.
"""

import os
import sys
from contextlib import ExitStack

import numpy as np

sys.path.insert(0, "/opt/trn_rl_repo")

import concourse.bass as bass  # noqa: E402
import concourse.tile as tile  # noqa: E402
from concourse import bacc, mybir  # noqa: E402

# Problem constants (hardcoded per contract)
N_BATCH = 8
C, H, W = 256, 64, 128
PAD = 4
D = 81  # 9x9 displacements
CH = 2  # c split into 2 K-halves of 128
HP, WP = H + 2 * PAD, W + 2 * PAD  # 72, 136

# Gram block geometry
BY, BX = 8, 16  # in1 block (M = BY*BX = 128)
WY, WX = BY + 2 * PAD, BX + 2 * PAD  # in2 window 16 x 24
NW = WY * WX  # 384 matmul N
NBY, NBX = H // BY, W // BX  # 8 x 8 = 64 blocks
NPAIR = NBY * (NBX // 2)  # 32 pair-tiles (2 blocks each)
NDST = 82  # scatter dst width per block (81 padded to even)

# Extraction split: device-scattered pairs vs host-extracted raw pairs
K_BAND = 21
K_RAW = NPAIR - K_BAND
OUT_W = K_BAND * 2 * NDST + K_RAW * 2 * NW

_CACHE = {}


def _bf16():
    import ml_dtypes

    return ml_dtypes.bfloat16


def _band_idx_table() -> np.ndarray:
    """Single-block table: idx[p, n] = d if Gram col n holds displacement d
    for partition p else -1.

    p = yhat*BX + xhat (in1 position in the 8x16 block)
    n = yw*WX + xw     (in2p position in the 16x24 window)
    valid: 0 <= yw-yhat <= 8 and 0 <= xw-xhat <= 8; d = (yw-yhat)*9+(xw-xhat)
    """
    idx = np.full((128, NW), -1, dtype=np.int16)
    for p in range(128):
        yh, xh = p // BX, p % BX
        for n in range(NW):
            yw, xw = n // WX, n % WX
            dyp, dxp = yw - yh, xw - xh
            if 0 <= dyp <= 8 and 0 <= dxp <= 8:
                idx[p, n] = dyp * 9 + dxp
    return idx


def _band_idx_table2() -> np.ndarray:
    """Two-block table [128, 2*NW]: second block's band lands at d+NDST."""
    t = _band_idx_table()
    t2 = np.where(t >= 0, t + NDST, t).astype(np.int16)
    return np.concatenate([t, t2], axis=1)


def _host_gather_idx() -> np.ndarray:
    """[128, 81] int64: Gram col (within one 384-col block window) holding
    displacement d for partition p."""
    gi = np.zeros((128, D), dtype=np.int64)
    for p in range(128):
        yh, xh = p // BX, p % BX
        for d in range(D):
            dyp, dxp = d // 9, d % 9
            gi[p, d] = (yh + dyp) * WX + (xh + dxp)
    return gi


def _retile_in1(a: np.ndarray) -> np.ndarray:
    """[C, H, W] f32 -> [C, NBY, NBX*BY*BX] bf16 with 8x16 blocks contiguous."""
    x = a.reshape(C, NBY, BY, NBX, BX)
    x = x.transpose(0, 1, 3, 2, 4)  # c, yb, xb, yhat, xhat
    return np.ascontiguousarray(
        x.reshape(C, NBY, NBX * BY * BX).astype(_bf16())
    )


def _pad_in2(a: np.ndarray) -> np.ndarray:
    """[C, H, W] f32 -> [C, H, WP] bf16 with the 4-col horizontal pad baked
    in (vertical pad rows are memset on device)."""
    out = np.zeros((C, H, WP), dtype=_bf16())
    out[:, :, PAD : PAD + W] = a.astype(_bf16())
    return out


def _build_kernel(ctx: ExitStack, tc: tile.TileContext, out, in1, in2, idx):
    nc = tc.nc
    f32 = mybir.dt.float32
    bf16 = mybir.dt.bfloat16
    i16 = mybir.dt.int16

    persist = ctx.enter_context(tc.tile_pool(name="persist", bufs=1))
    # in1 block-contiguous: [c, h, yb, (xb, yhat, xhat)]
    in1_sb = persist.tile([128, CH, NBY, NBX * BY * BX], bf16, tag="in1_sb")
    in2_sb = persist.tile([128, CH, HP, WP], bf16, tag="in2_sb")
    idx_sb = persist.tile([128, 2 * NW], i16, tag="idx_sb")
    band_all = persist.tile([128, K_BAND * 2 * NDST], bf16, tag="band_all")
    raw_all = persist.tile([128, K_RAW * 2 * NW], bf16, tag="raw_all")

    # --- load inputs (fine-grained, interleaved, so compute starts early) ---
    # zero only the vertical pad rows of in2_sb (horizontal pad ships zeroed)
    nc.vector.memset(in2_sb[:, :, 0:PAD, :].bitcast(f32), 0.0)
    nc.vector.memset(in2_sb[:, :, PAD + H : HP, :].bitcast(f32), 0.0)

    def load_in1(yb, n=1):
        for h in range(CH):
            cs = slice(h * 128, (h + 1) * 128)
            nc.sync.dma_start(
                in1_sb[:, h, yb : yb + n, :], in1[cs, yb : yb + n, :]
            )

    def load_in2(r0, n):  # n-row chunk starting at source row r0
        for h in range(CH):
            cs = slice(h * 128, (h + 1) * 128)
            nc.sync.dma_start(
                in2_sb[:, h, PAD + r0 : PAD + r0 + n, :],
                in2[cs, r0 : r0 + n, :],
            )

    # pair yb=0 needs in1 row 0 + in2 rows 0-11; idx before the 1st scatter
    load_in1(0)
    load_in2(0, 12)
    nc.sync.dma_start(idx_sb[:], idx[:])
    load_in2(12, 12)
    load_in1(1)
    load_in1(2, 2)
    load_in2(24, 16)
    load_in1(4, 2)
    load_in2(40, 8)
    load_in2(48, 12)
    load_in1(6, 2)
    load_in2(60, 4)

    ps_pool = ctx.enter_context(tc.tile_pool(name="ps", bufs=8, space="PSUM"))
    # one gsb buffer per band pair: the Pool scatter backlog must never
    # throttle evacuation (and through PSUM recycling, the PE itself)
    gsb_pool = ctx.enter_context(tc.tile_pool(name="gsb", bufs=K_BAND))

    inv_c = 1.0 / C

    # raw store groups: (first raw q, one-past-last raw q), issued right
    # after the last constituent pair's evacuation
    # last raw pair ships as two half-pair stores right after each block's
    # evac (shortens the tail by one evacuation). Groups are emitted one
    # pair LATE (at q == hi) so the store's wait on the other engine's evac
    # is already satisfied and never head-of-line blocks Act's evac queue.
    raw_groups = [(0, 2), (2, 4), (4, 6), (6, 8), (8, 9), (9, 10)]
    band_chunks = [(0, 8), (8, 13), (13, 17), (17, 19), (19, K_BAND)]

    for yb in range(NBY):
        y0 = yb * BY
        for xp in range(NBX // 2):  # xb pairs
            pair = yb * (NBX // 2) + xp
            if pair < K_BAND:
                gt = gsb_pool.tile([128, 2 * NW], bf16, tag="gsb")
                g = gt[:]
            else:
                q = pair - K_BAND
                g = raw_all[:, q * 2 * NW : (q + 1) * 2 * NW]
            bd = (
                band_all[:, pair * 2 * NDST : (pair + 1) * 2 * NDST]
                if pair < K_BAND
                else None
            )
            for j in range(2):
                xb = 2 * xp + j
                x0 = xb * BX
                ps = ps_pool.tile([128, 512], f32, tag="ps")  # 1 PSUM bank
                for h in range(CH):
                    lhsT = in1_sb[:, h, yb, xb * 128 : (xb + 1) * 128]
                    rhs = in2_sb[:, h, y0 : y0 + WY, x0 : x0 + WX]
                    nc.tensor.matmul(
                        ps[:, 0:NW],
                        lhsT,
                        rhs,
                        start=(h == 0),
                        stop=(h == CH - 1),
                    )
                # per-block evac: fused 1/C scale + bf16 downcast, alternating
                # ScalarE / VectorE; frees the PSUM bank immediately so the
                # scatter backlog never throttles the PE
                gj = g[:, j * NW : (j + 1) * NW]
                if (2 * pair + j) % 2 == 0:
                    nc.scalar.mul(gj, ps[:, 0:NW], inv_c)
                else:
                    nc.vector.tensor_scalar(
                        gj, ps[:, 0:NW], inv_c, None, mybir.AluOpType.mult
                    )
                if pair == 0:
                    # warm-start: per-block scatters so Pool starts earlier
                    nc.gpsimd.local_scatter(
                        bd[:, j * NDST : (j + 1) * NDST], gj,
                        idx_sb[:, 0:NW], 128, NDST, NW,
                    )
                elif pair == NPAIR - 1:
                    # tail: ship each half as soon as its evac lands
                    q = pair - K_BAND
                    o0 = K_BAND * 2 * NDST + q * 2 * NW
                    nc.scalar.dma_start(
                        out[:, o0 + j * NW : o0 + (j + 1) * NW],
                        raw_all[:, q * 2 * NW + j * NW : q * 2 * NW + (j + 1) * NW],
                    )
            if pair == 0:
                continue
            if pair < K_BAND:
                # band extraction on GpSimdE via per-partition index table
                nc.gpsimd.local_scatter(
                    bd, g, idx_sb[:], 128, 2 * NDST, 2 * NW
                )
            else:
                # raw Gram pair: host extracts; grouped stores on the
                # Activation DMA queue (decoupled from Pool-gated band stores)
                q = pair - K_BAND
                for lo, hi in raw_groups:
                    if q == hi:
                        o0 = K_BAND * 2 * NDST + lo * 2 * NW
                        o1 = K_BAND * 2 * NDST + hi * 2 * NW
                        nc.scalar.dma_start(
                            out[:, o0:o1],
                            raw_all[:, lo * 2 * NW : hi * 2 * NW],
                        )
    # band stores on SP after the loads: their Pool-gated waits block
    # nothing else
    for lo, hi in band_chunks:
        nc.sync.dma_start(
            out[:, lo * 2 * NDST : hi * 2 * NDST],
            band_all[:, lo * 2 * NDST : hi * 2 * NDST],
        )


def _get_nc():
    if "nc" in _CACHE:
        return _CACHE["nc"]
    nc = bacc.Bacc(
        "TRN2",
        target_bir_lowering=False,
        debug=False,
        num_devices=N_BATCH,
    )
    in1 = nc.dram_tensor(
        "input1", [C, NBY, NBX * BY * BX], mybir.dt.bfloat16,
        kind="ExternalInput"
    ).ap()
    in2 = nc.dram_tensor(
        "input2", [C, H, WP], mybir.dt.bfloat16, kind="ExternalInput"
    ).ap()
    idx = nc.dram_tensor(
        "band_idx", [128, 2 * NW], mybir.dt.int16, kind="ExternalInput"
    ).ap()
    out = nc.dram_tensor(
        "out", [128, OUT_W], mybir.dt.bfloat16, kind="ExternalOutput"
    ).ap()
    with tile.TileContext(nc) as tc:
        with ExitStack() as ctx:
            _build_kernel(ctx, tc, out, in1, in2, idx)
    nc.compile()
    _CACHE["nc"] = nc
    return nc


def _make_executor():
    """Build a jitted shard_map executor over the 8 cores (fresh per call —
    re-executing a loaded NEFF has a stale-state hazard on this stack)."""
    import jax
    from jax.experimental.shard_map import shard_map
    from jax.sharding import Mesh, PartitionSpec

    from concourse import bass2jax

    nc = _get_nc()
    bass2jax.install_neuronx_cc_hook()
    assert nc.dbg_addr is None
    partition_name = (
        nc.partition_id_tensor.name if nc.partition_id_tensor else None
    )

    in_names, out_names, out_avals, zero_outs = [], [], [], []
    for alloc in nc.m.functions[0].allocations:
        if not isinstance(alloc, mybir.MemoryLocationSet):
            continue
        name = alloc.memorylocations[0].name
        if alloc.kind == "ExternalInput":
            if name != partition_name:
                in_names.append(name)
        elif alloc.kind == "ExternalOutput":
            out_names.append(name)
            shape = tuple(alloc.tensor_shape)
            dtype = mybir.dt.np(alloc.dtype)
            out_avals.append(jax.core.ShapedArray(shape, dtype))
            zero_outs.append(np.zeros(shape, dtype))
    n_params = len(in_names)
    in_names_full = tuple(in_names + out_names)
    if partition_name is not None:
        in_names_full = in_names_full + (partition_name,)

    def _body(*args):
        operands = list(args)
        if partition_name is not None:
            operands.append(bass2jax.partition_id_tensor())
        outs = bass2jax._bass_exec_p.bind(
            *operands,
            out_avals=tuple(out_avals),
            in_names=in_names_full,
            out_names=tuple(out_names),
            lowering_input_output_aliases=(),
            sim_require_finite=True,
            sim_require_nnan=True,
            nc=nc,
        )
        return tuple(outs)

    devices = jax.devices()[:N_BATCH]
    mesh = Mesh(np.asarray(devices), ("core",))
    nio = n_params + len(out_names)
    sharded = jax.jit(
        shard_map(
            _body,
            mesh=mesh,
            in_specs=(PartitionSpec("core"),) * nio,
            out_specs=(PartitionSpec("core"),) * len(out_names),
            check_rep=False,
        ),
        donate_argnums=tuple(range(n_params, nio)),
        keep_unused=True,
    )
    return (sharded, in_names, out_names, out_avals, zero_outs, mesh)


def _get_executor(fresh: bool = False):
    if fresh or "exec" not in _CACHE:
        _CACHE["exec"] = _make_executor()
    return _CACHE["exec"]


def _concat_inputs(in_maps):
    _, in_names, *_ = _get_executor()
    return [
        np.concatenate([np.asarray(m[name]) for m in in_maps], axis=0)
        for name in in_names
    ]


def _run_concat(concat_in):
    import jax

    sharded, in_names, out_names, out_avals, zero_outs, mesh = _get_executor()
    concat_zeros = [
        np.zeros((N_BATCH * z.shape[0], *z.shape[1:]), z.dtype) for z in zero_outs
    ]
    out_arrs = sharded(*concat_in, *concat_zeros)
    jax.block_until_ready(out_arrs)
    return {
        name: np.asarray(out_arrs[i]).reshape(N_BATCH, *out_avals[i].shape)
        for i, name in enumerate(out_names)
    }


def _unpack_out(raw: np.ndarray) -> np.ndarray:
    """[N, 128, OUT_W] bf16 -> [N, 81, 64, 128] f32.

    Band region: raw[n, p, pair*192 + j*96 + d] for pair < K_BAND.
    Raw region:  raw[n, p, K_BAND*192 + q*768 + j*384 + w] holds the full
    Gram window; host gathers w = (yh+dy)*24 + (xh+dx).
    """
    r = raw.astype(np.float32)
    band = r[:, :, : K_BAND * 2 * NDST].reshape(N_BATCH, 128, K_BAND, 2, NDST)
    band = band[:, :, :, :, :D]  # [N, p, pair, j, 81]
    g = r[:, :, K_BAND * 2 * NDST :].reshape(N_BATCH, 128, K_RAW, 2, NW)
    gi = _host_gather_idx()  # [128, 81]
    rawband = np.take_along_axis(
        g, gi[None, :, None, None, :], axis=4
    )  # [N, p, q, j, 81]
    full = np.concatenate([band, rawband], axis=2)  # [N, p, 32, j, 81]
    full = full.reshape(N_BATCH, BY, BX, NBY, NBX // 2, 2, D)
    # (n, yhat, xhat, yb, xp, j, d) -> (n, d, yb, yhat, xp, j, xhat)
    full = full.transpose(0, 6, 3, 1, 4, 5, 2)
    return np.ascontiguousarray(full.reshape(N_BATCH, D, H, W))


def kernel(input1: np.ndarray, input2: np.ndarray) -> np.ndarray:
    assert input1.shape == (N_BATCH, C, H, W), input1.shape
    idx_np = _band_idx_table2()
    in_maps = [
        {
            "input1": _retile_in1(np.asarray(input1[i], dtype=np.float32)),
            "input2": _pad_in2(np.asarray(input2[i], dtype=np.float32)),
            "band_idx": idx_np,
        }
        for i in range(N_BATCH)
    ]
    # Fresh executor per call: re-executing an already-loaded NEFF produced
    # stale-state corruption on this stack; a fresh load is always clean.
    _get_executor(fresh=True)
    concat_in = _concat_inputs(in_maps)
    _CACHE["last_concat_in"] = concat_in
    outs = _run_concat(concat_in)
    return _unpack_out(outs["out"])


def time_exec_ns(reps: int = 5):
    """Best-of-N wall time of the sharded device execution, in ns.

    Caveat: no NTFF profiling is available under axon in this container, so
    this includes the PJRT/axon dispatch round-trip (~70ms floor) and vastly
    overstates on-device kernel time.
    """
    import time

    import jax
    from jax.sharding import NamedSharding, PartitionSpec

    sharded, in_names, out_names, out_avals, zero_outs, mesh = _get_executor()
    concat_in = _CACHE.get("last_concat_in")
    if concat_in is None:
        return None
    sh = NamedSharding(mesh, PartitionSpec("core"))
    dev_in = [jax.device_put(a, sh) for a in concat_in]
    jax.block_until_ready(dev_in)
    best = None
    for _ in range(reps):
        concat_zeros = [
            jax.device_put(
                np.zeros((N_BATCH * z.shape[0], *z.shape[1:]), z.dtype), sh
            )
            for z in zero_outs
        ]
        jax.block_until_ready(concat_zeros)
        t0 = time.perf_counter()
        out_arrs = sharded(*dev_in, *concat_zeros)
        jax.block_until_ready(out_arrs)
        dt = time.perf_counter() - t0
        best = dt if best is None else min(best, dt)
    return int(best * 1e9)


# revision 52
# speedup vs baseline: 1.0158x; 1.0158x over previous
"""FlowNet-C correlation (max_displacement=4) on 8 Trainium2 NeuronCores.

Strategy: data-parallel over batch N=8 (one sample per core).
Per core: out[d=(dy,dx), y, x] = 1/C * sum_c in1[c,y,x] * in2pad[c,y+dy,x+dx]

Mapping: the correlation is a banded Gram. For each 8x16 spatial block of
in1 (M=128 positions, host-pre-tiled to be SBUF-contiguous) we matmul
(contract c=256 in 2 K-halves) against the 16x24 padded window of in2
(N=384 columns) in bf16 (full-rate; inputs are bf16-converted on host,
in2 horizontally pre-padded to 136 cols so every DMA chunk is >=512B
contiguous). Two adjacent blocks share a 2-bank PSUM tile; ScalarE /
VectorE (alternating) evacuate with a fused 1/C scale + bf16 downcast.
Band extraction (81 of 384 Gram cols per position, per-partition offsets)
is split: the first K_BAND pair-tiles go through GPSIMD local_scatter on
device; the last K_RAW pairs ship their raw Gram tile and the host
extracts, balancing the Pool engine against the serial DMA resource.
"""

import os
import sys
from contextlib import ExitStack

import numpy as np

sys.path.insert(0, "/opt/trn_rl_repo")

import concourse.bass as bass  # noqa: E402
import concourse.tile as tile  # noqa: E402
from concourse import bacc, mybir  # noqa: E402

# Problem constants (hardcoded per contract)
N_BATCH = 8
C, H, W = 256, 64, 128
PAD = 4
D = 81  # 9x9 displacements
CH = 2  # c split into 2 K-halves of 128
HP, WP = H + 2 * PAD, W + 2 * PAD  # 72, 136

# Gram block geometry
BY, BX = 8, 16  # in1 block (M = BY*BX = 128)
WY, WX = BY + 2 * PAD, BX + 2 * PAD  # in2 window 16 x 24
NW = WY * WX  # 384 matmul N
NBY, NBX = H // BY, W // BX  # 8 x 8 = 64 blocks
NPAIR = NBY * (NBX // 2)  # 32 pair-tiles (2 blocks each)
NDST = 82  # scatter dst width per block (81 padded to even)

# Extraction split: device-scattered pairs vs host-extracted raw pairs
K_BAND = 21
K_RAW = NPAIR - K_BAND
OUT_W = K_BAND * 2 * NDST + K_RAW * 2 * NW

_CACHE = {}


def _bf16():
    import ml_dtypes

    return ml_dtypes.bfloat16


def _band_idx_table() -> np.ndarray:
    """Single-block table: idx[p, n] = d if Gram col n holds displacement d
    for partition p else -1.

    p = yhat*BX + xhat (in1 position in the 8x16 block)
    n = yw*WX + xw     (in2p position in the 16x24 window)
    valid: 0 <= yw-yhat <= 8 and 0 <= xw-xhat <= 8; d = (yw-yhat)*9+(xw-xhat)
    """
    idx = np.full((128, NW), -1, dtype=np.int16)
    for p in range(128):
        yh, xh = p // BX, p % BX
        for n in range(NW):
            yw, xw = n // WX, n % WX
            dyp, dxp = yw - yh, xw - xh
            if 0 <= dyp <= 8 and 0 <= dxp <= 8:
                idx[p, n] = dyp * 9 + dxp
    return idx


def _band_idx_table2() -> np.ndarray:
    """Two-block table [128, 2*NW]: second block's band lands at d+NDST."""
    t = _band_idx_table()
    t2 = np.where(t >= 0, t + NDST, t).astype(np.int16)
    return np.concatenate([t, t2], axis=1)


def _host_gather_idx() -> np.ndarray:
    """[128, 81] int64: Gram col (within one 384-col block window) holding
    displacement d for partition p."""
    gi = np.zeros((128, D), dtype=np.int64)
    for p in range(128):
        yh, xh = p // BX, p % BX
        for d in range(D):
            dyp, dxp = d // 9, d % 9
            gi[p, d] = (yh + dyp) * WX + (xh + dxp)
    return gi


def _retile_in1(a: np.ndarray) -> np.ndarray:
    """[C, H, W] f32 -> [C, NBY, NBX*BY*BX] bf16 with 8x16 blocks contiguous."""
    x = a.reshape(C, NBY, BY, NBX, BX)
    x = x.transpose(0, 1, 3, 2, 4)  # c, yb, xb, yhat, xhat
    return np.ascontiguousarray(
        x.reshape(C, NBY, NBX * BY * BX).astype(_bf16())
    )


def _pad_in2(a: np.ndarray) -> np.ndarray:
    """[C, H, W] f32 -> [C, H, WP] bf16 with the 4-col horizontal pad baked
    in (vertical pad rows are memset on device)."""
    out = np.zeros((C, H, WP), dtype=_bf16())
    out[:, :, PAD : PAD + W] = a.astype(_bf16())
    return out


def _build_kernel(ctx: ExitStack, tc: tile.TileContext, out, in1, in2, idx):
    nc = tc.nc
    f32 = mybir.dt.float32
    bf16 = mybir.dt.bfloat16
    i16 = mybir.dt.int16

    persist = ctx.enter_context(tc.tile_pool(name="persist", bufs=1))
    # in1 block-contiguous: [c, h, yb, (xb, yhat, xhat)]
    in1_sb = persist.tile([128, CH, NBY, NBX * BY * BX], bf16, tag="in1_sb")
    in2_sb = persist.tile([128, CH, HP, WP], bf16, tag="in2_sb")
    idx_sb = persist.tile([128, 2 * NW], i16, tag="idx_sb")
    band_all = persist.tile([128, K_BAND * 2 * NDST], bf16, tag="band_all")
    raw_all = persist.tile([128, K_RAW * 2 * NW], bf16, tag="raw_all")

    # --- load inputs (fine-grained, interleaved, so compute starts early) ---
    # zero only the vertical pad rows of in2_sb (horizontal pad ships zeroed)
    nc.vector.memset(in2_sb[:, :, 0:PAD, :].bitcast(f32), 0.0)
    nc.vector.memset(in2_sb[:, :, PAD + H : HP, :].bitcast(f32), 0.0)

    def load_in1(yb, n=1):
        for h in range(CH):
            cs = slice(h * 128, (h + 1) * 128)
            nc.sync.dma_start(
                in1_sb[:, h, yb : yb + n, :], in1[cs, yb : yb + n, :]
            )

    def load_in2(r0, n):  # n-row chunk starting at source row r0
        for h in range(CH):
            cs = slice(h * 128, (h + 1) * 128)
            nc.sync.dma_start(
                in2_sb[:, h, PAD + r0 : PAD + r0 + n, :],
                in2[cs, r0 : r0 + n, :],
            )

    # pair yb=0 needs in1 row 0 + in2 rows 0-11; idx before the 1st scatter
    load_in1(0)
    load_in2(0, 12)
    nc.sync.dma_start(idx_sb[:], idx[:])
    load_in2(12, 12)
    load_in1(1)
    load_in1(2, 2)
    load_in2(24, 16)
    load_in1(4, 2)
    load_in2(40, 8)
    load_in2(48, 12)
    load_in1(6, 2)
    load_in2(60, 4)

    ps_pool = ctx.enter_context(tc.tile_pool(name="ps", bufs=8, space="PSUM"))
    # one gsb buffer per band pair: the Pool scatter backlog must never
    # throttle evacuation (and through PSUM recycling, the PE itself)
    gsb_pool = ctx.enter_context(tc.tile_pool(name="gsb", bufs=K_BAND))

    inv_c = 1.0 / C

    # raw store groups: (first raw q, one-past-last raw q), issued right
    # after the last constituent pair's evacuation
    # last raw pair ships as two half-pair stores right after each block's
    # evac (shortens the tail by one evacuation). Groups are emitted one
    # pair LATE (at q == hi) so the store's wait on the other engine's evac
    # is already satisfied and never head-of-line blocks Act's evac queue.
    raw_groups = [(0, 2), (2, 4), (4, 6), (6, 8), (8, 10)]
    band_chunks = [(0, 8), (8, 13), (13, 17), (17, 19), (19, K_BAND)]

    for yb in range(NBY):
        y0 = yb * BY
        for xp in range(NBX // 2):  # xb pairs
            pair = yb * (NBX // 2) + xp
            if pair < K_BAND:
                gt = gsb_pool.tile([128, 2 * NW], bf16, tag="gsb")
                g = gt[:]
            else:
                q = pair - K_BAND
                g = raw_all[:, q * 2 * NW : (q + 1) * 2 * NW]
            bd = (
                band_all[:, pair * 2 * NDST : (pair + 1) * 2 * NDST]
                if pair < K_BAND
                else None
            )
            for j in range(2):
                xb = 2 * xp + j
                x0 = xb * BX
                ps = ps_pool.tile([128, 512], f32, tag="ps")  # 1 PSUM bank
                for h in range(CH):
                    lhsT = in1_sb[:, h, yb, xb * 128 : (xb + 1) * 128]
                    rhs = in2_sb[:, h, y0 : y0 + WY, x0 : x0 + WX]
                    nc.tensor.matmul(
                        ps[:, 0:NW],
                        lhsT,
                        rhs,
                        start=(h == 0),
                        stop=(h == CH - 1),
                    )
                # per-block evac: fused 1/C scale + bf16 downcast, alternating
                # ScalarE / VectorE; frees the PSUM bank immediately so the
                # scatter backlog never throttles the PE
                gj = g[:, j * NW : (j + 1) * NW]
                if (2 * pair + j) % 2 == 0:
                    nc.scalar.mul(gj, ps[:, 0:NW], inv_c)
                else:
                    nc.vector.tensor_scalar(
                        gj, ps[:, 0:NW], inv_c, None, mybir.AluOpType.mult
                    )
                if pair == 0:
                    # warm-start: per-block scatters so Pool starts earlier
                    nc.gpsimd.local_scatter(
                        bd[:, j * NDST : (j + 1) * NDST], gj,
                        idx_sb[:, 0:NW], 128, NDST, NW,
                    )
                elif pair == NPAIR - 1:
                    # tail: ship each half as soon as its evac lands
                    q = pair - K_BAND
                    o0 = K_BAND * 2 * NDST + q * 2 * NW
                    nc.scalar.dma_start(
                        out[:, o0 + j * NW : o0 + (j + 1) * NW],
                        raw_all[:, q * 2 * NW + j * NW : q * 2 * NW + (j + 1) * NW],
                    )
            if pair == 0:
                continue
            if pair < K_BAND:
                # band extraction on GpSimdE via per-partition index table
                nc.gpsimd.local_scatter(
                    bd, g, idx_sb[:], 128, 2 * NDST, 2 * NW
                )
            else:
                # raw Gram pair: host extracts; grouped stores on the
                # Activation DMA queue (decoupled from Pool-gated band stores)
                q = pair - K_BAND
                for lo, hi in raw_groups:
                    if q == hi:
                        o0 = K_BAND * 2 * NDST + lo * 2 * NW
                        o1 = K_BAND * 2 * NDST + hi * 2 * NW
                        nc.scalar.dma_start(
                            out[:, o0:o1],
                            raw_all[:, lo * 2 * NW : hi * 2 * NW],
                        )
    # band stores on SP after the loads: their Pool-gated waits block
    # nothing else
    for lo, hi in band_chunks:
        nc.sync.dma_start(
            out[:, lo * 2 * NDST : hi * 2 * NDST],
            band_all[:, lo * 2 * NDST : hi * 2 * NDST],
        )


def _get_nc():
    if "nc" in _CACHE:
        return _CACHE["nc"]
    nc = bacc.Bacc(
        "TRN2",
        target_bir_lowering=False,
        debug=False,
        num_devices=N_BATCH,
    )
    in1 = nc.dram_tensor(
        "input1", [C, NBY, NBX * BY * BX], mybir.dt.bfloat16,
        kind="ExternalInput"
    ).ap()
    in2 = nc.dram_tensor(
        "input2", [C, H, WP], mybir.dt.bfloat16, kind="ExternalInput"
    ).ap()
    idx = nc.dram_tensor(
        "band_idx", [128, 2 * NW], mybir.dt.int16, kind="ExternalInput"
    ).ap()
    out = nc.dram_tensor(
        "out", [128, OUT_W], mybir.dt.bfloat16, kind="ExternalOutput"
    ).ap()
    with tile.TileContext(nc) as tc:
        with ExitStack() as ctx:
            _build_kernel(ctx, tc, out, in1, in2, idx)
    nc.compile()
    _CACHE["nc"] = nc
    return nc


def _make_executor():
    """Build a jitted shard_map executor over the 8 cores (fresh per call —
    re-executing a loaded NEFF has a stale-state hazard on this stack)."""
    import jax
    from jax.experimental.shard_map import shard_map
    from jax.sharding import Mesh, PartitionSpec

    from concourse import bass2jax

    nc = _get_nc()
    bass2jax.install_neuronx_cc_hook()
    assert nc.dbg_addr is None
    partition_name = (
        nc.partition_id_tensor.name if nc.partition_id_tensor else None
    )

    in_names, out_names, out_avals, zero_outs = [], [], [], []
    for alloc in nc.m.functions[0].allocations:
        if not isinstance(alloc, mybir.MemoryLocationSet):
            continue
        name = alloc.memorylocations[0].name
        if alloc.kind == "ExternalInput":
            if name != partition_name:
                in_names.append(name)
        elif alloc.kind == "ExternalOutput":
            out_names.append(name)
            shape = tuple(alloc.tensor_shape)
            dtype = mybir.dt.np(alloc.dtype)
            out_avals.append(jax.core.ShapedArray(shape, dtype))
            zero_outs.append(np.zeros(shape, dtype))
    n_params = len(in_names)
    in_names_full = tuple(in_names + out_names)
    if partition_name is not None:
        in_names_full = in_names_full + (partition_name,)

    def _body(*args):
        operands = list(args)
        if partition_name is not None:
            operands.append(bass2jax.partition_id_tensor())
        outs = bass2jax._bass_exec_p.bind(
            *operands,
            out_avals=tuple(out_avals),
            in_names=in_names_full,
            out_names=tuple(out_names),
            lowering_input_output_aliases=(),
            sim_require_finite=True,
            sim_require_nnan=True,
            nc=nc,
        )
        return tuple(outs)

    devices = jax.devices()[:N_BATCH]
    mesh = Mesh(np.asarray(devices), ("core",))
    nio = n_params + len(out_names)
    sharded = jax.jit(
        shard_map(
            _body,
            mesh=mesh,
            in_specs=(PartitionSpec("core"),) * nio,
            out_specs=(PartitionSpec("core"),) * len(out_names),
            check_rep=False,
        ),
        donate_argnums=tuple(range(n_params, nio)),
        keep_unused=True,
    )
    return (sharded, in_names, out_names, out_avals, zero_outs, mesh)


def _get_executor(fresh: bool = False):
    if fresh or "exec" not in _CACHE:
        _CACHE["exec"] = _make_executor()
    return _CACHE["exec"]


def _concat_inputs(in_maps):
    _, in_names, *_ = _get_executor()
    return [
        np.concatenate([np.asarray(m[name]) for m in in_maps], axis=0)
        for name in in_names
    ]


def _run_concat(concat_in):
    import jax

    sharded, in_names, out_names, out_avals, zero_outs, mesh = _get_executor()
    concat_zeros = [
        np.zeros((N_BATCH * z.shape[0], *z.shape[1:]), z.dtype) for z in zero_outs
    ]
    out_arrs = sharded(*concat_in, *concat_zeros)
    jax.block_until_ready(out_arrs)
    return {
        name: np.asarray(out_arrs[i]).reshape(N_BATCH, *out_avals[i].shape)
        for i, name in enumerate(out_names)
    }


def _unpack_out(raw: np.ndarray) -> np.ndarray:
    """[N, 128, OUT_W] bf16 -> [N, 81, 64, 128] f32.

    Band region: raw[n, p, pair*192 + j*96 + d] for pair < K_BAND.
    Raw region:  raw[n, p, K_BAND*192 + q*768 + j*384 + w] holds the full
    Gram window; host gathers w = (yh+dy)*24 + (xh+dx).
    """
    r = raw.astype(np.float32)
    band = r[:, :, : K_BAND * 2 * NDST].reshape(N_BATCH, 128, K_BAND, 2, NDST)
    band = band[:, :, :, :, :D]  # [N, p, pair, j, 81]
    g = r[:, :, K_BAND * 2 * NDST :].reshape(N_BATCH, 128, K_RAW, 2, NW)
    gi = _host_gather_idx()  # [128, 81]
    rawband = np.take_along_axis(
        g, gi[None, :, None, None, :], axis=4
    )  # [N, p, q, j, 81]
    full = np.concatenate([band, rawband], axis=2)  # [N, p, 32, j, 81]
    full = full.reshape(N_BATCH, BY, BX, NBY, NBX // 2, 2, D)
    # (n, yhat, xhat, yb, xp, j, d) -> (n, d, yb, yhat, xp, j, xhat)
    full = full.transpose(0, 6, 3, 1, 4, 5, 2)
    return np.ascontiguousarray(full.reshape(N_BATCH, D, H, W))


def kernel(input1: np.ndarray, input2: np.ndarray) -> np.ndarray:
    assert input1.shape == (N_BATCH, C, H, W), input1.shape
    idx_np = _band_idx_table2()
    in_maps = [
        {
            "input1": _retile_in1(np.asarray(input1[i], dtype=np.float32)),
            "input2": _pad_in2(np.asarray(input2[i], dtype=np.float32)),
            "band_idx": idx_np,
        }
        for i in range(N_BATCH)
    ]
    # Fresh executor per call: re-executing an already-loaded NEFF produced
    # stale-state corruption on this stack; a fresh load is always clean.
    _get_executor(fresh=True)
    concat_in = _concat_inputs(in_maps)
    _CACHE["last_concat_in"] = concat_in
    outs = _run_concat(concat_in)
    return _unpack_out(outs["out"])


def time_exec_ns(reps: int = 5):
    """Best-of-N wall time of the sharded device execution, in ns.

    Caveat: no NTFF profiling is available under axon in this container, so
    this includes the PJRT/axon dispatch round-trip (~70ms floor) and vastly
    overstates on-device kernel time.
    """
    import time

    import jax
    from jax.sharding import NamedSharding, PartitionSpec

    sharded, in_names, out_names, out_avals, zero_outs, mesh = _get_executor()
    concat_in = _CACHE.get("last_concat_in")
    if concat_in is None:
        return None
    sh = NamedSharding(mesh, PartitionSpec("core"))
    dev_in = [jax.device_put(a, sh) for a in concat_in]
    jax.block_until_ready(dev_in)
    best = None
    for _ in range(reps):
        concat_zeros = [
            jax.device_put(
                np.zeros((N_BATCH * z.shape[0], *z.shape[1:]), z.dtype), sh
            )
            for z in zero_outs
        ]
        jax.block_until_ready(concat_zeros)
        t0 = time.perf_counter()
        out_arrs = sharded(*dev_in, *concat_zeros)
        jax.block_until_ready(out_arrs)
        dt = time.perf_counter() - t0
        best = dt if best is None else min(best, dt)
    return int(best * 1e9)
